# revision 8
# baseline (speedup 1.0000x reference)
"""Multi-head attention + output projection, sharded over 8 TRN2 NeuronCores.

Problem: Q,K,V [4,1024,1024] f32; 16 heads x 64 dim; softmax(QK^T/sqrt(1024))V,
concat heads, out @ W_H.T + b_H.

Sharding: 8 cores = 4 batch x 2 query-halves. Each core computes full attention
(all 16 heads, all 1024 keys) for its 512 queries plus the output projection for
those rows. Output rows are disjoint -> no collectives.

Per-core kernel (all matmuls float32r: full PE rate, ~1.6e-4 matmul precision;
fp32 PSUM accumulation). Heads are processed in even/odd pairs whose QK^T
matmuls use complementary PE row groups (K=64 each at partition offsets 0/64)
so they can overlap in the array:
  scoresT[k,q] = KhT.T @ QhT   (contract d=64; Q pre-scaled by 1/sqrt(D) on host)
  expT = exp(scoresT)          (ACT, psum->sbuf, f32r; no max-subtraction:
                                |scores/32| <= ~1.5 for N(0,1) inputs)
  outT_aug[65,q] = sum_k V_aug[k,65].T @ expT[k,q]   (V has a ones column ->
                                row 64 = softmax denominator)
  normalize: DVE reciprocal + gpsimd partition_broadcast + DVE multiply
  final[q,n] = outT_norm.T @ W_H.T + b_H   (contract over 1024 = 8 chunks)
"""
import sys
import os

sys.path.insert(0, "/opt/trn_rl_repo")

import numpy as np

B, L, D, H, HD = 4, 1024, 1024, 16, 64
NCORES = 8
QBLK = L // 2  # 512 queries per core
SCALE = 1.0 / np.sqrt(np.float32(D))

_STATE = {}


def _build_nc(niter=1):
    import concourse.bass as bass
    import concourse.tile as tile
    from concourse import bacc, mybir
    from contextlib import ExitStack

    F32 = mybir.dt.float32
    F32R = mybir.dt.float32r
    Exp = mybir.ActivationFunctionType.Exp

    nc = bacc.Bacc("TRN2", target_bir_lowering=False, debug=False)
    qt = nc.dram_tensor("qt", [128, 8, QBLK], F32R, kind="ExternalInput")
    kt = nc.dram_tensor("kt", [128, 8, L], F32R, kind="ExternalInput")
    vv = nc.dram_tensor("vv", [128, H, 8, HD + 1], F32R, kind="ExternalInput")
    wht = nc.dram_tensor("wht", [128, 8, D], F32R, kind="ExternalInput")
    bias = nc.dram_tensor("bias", [128, D], F32, kind="ExternalInput")
    out = nc.dram_tensor("out", [QBLK, D], F32, kind="ExternalOutput")

    with tile.TileContext(nc) as tc, ExitStack() as ctx:
        singles = ctx.enter_context(tc.tile_pool(name="singles", bufs=1))
        qk_pool = ctx.enter_context(tc.tile_pool(name="qk", bufs=2))
        v_pool = ctx.enter_context(tc.tile_pool(name="vp", bufs=4))
        exp_pool = ctx.enter_context(tc.tile_pool(name="exp", bufs=2))
        norm_pool = ctx.enter_context(tc.tile_pool(name="norm", bufs=4))
        final_pool = ctx.enter_context(tc.tile_pool(name="final", bufs=2))
        scps = ctx.enter_context(tc.tile_pool(name="scps", bufs=2, space="PSUM"))
        ov_ps = ctx.enter_context(tc.tile_pool(name="ovps", bufs=2, space="PSUM"))
        proj_ps = ctx.enter_context(tc.tile_pool(name="prps", bufs=2, space="PSUM"))

        def body(_=None):
            # warm the exp table while DMAs run
            warm_in = singles.tile([1, 8], F32, tag="warm_in")
            warm_out = singles.tile([1, 8], F32, tag="warm_out")
            nc.vector.memset(warm_in, 0.0)
            nc.scalar.activation(out=warm_out, in_=warm_in, func=Exp)

            sb_bias = singles.tile([128, D], F32, tag="bias")
            nc.sync.dma_start(sb_bias, bias.ap())

            # normalized concatenated attention output, transposed: [hd, q]
            outT = singles.tile([128, 8, QBLK], F32R, tag="outT")

            for pj in range(8):
                h0, h1 = 2 * pj, 2 * pj + 1
                qt_t = qk_pool.tile([128, QBLK], F32R, tag="qt")
                nc.sync.dma_start(qt_t, qt.ap()[:, pj])
                kt_t = qk_pool.tile([128, L], F32R, tag="kt")
                nc.sync.dma_start(kt_t, kt.ap()[:, pj])
                v0 = v_pool.tile([128, 8, HD + 1], F32R, tag="v")
                nc.sync.dma_start(v0, vv.ap()[:, h0])
                v1 = v_pool.tile([128, 8, HD + 1], F32R, tag="v")
                nc.sync.dma_start(v1, vv.ap()[:, h1])

                # expT[p, c, par, q]
                expT = exp_pool.tile([128, 8, 2, QBLK], F32R, tag="expT")
                ov0 = ov_ps.tile([HD + 1, QBLK], F32, tag="ov")
                ov1 = ov_ps.tile([HD + 1, QBLK], F32, tag="ov")

                def attnv(c):
                    nc.tensor.matmul(
                        ov0[:, :], lhsT=v0[:, c, :], rhs=expT[:, c, 0, :],
                        start=(c == 0), stop=(c == 7))
                    nc.tensor.matmul(
                        ov1[:, :], lhsT=v1[:, c, :], rhs=expT[:, c, 1, :],
                        start=(c == 0), stop=(c == 7))

                for c in range(8):
                    S = scps.tile([128, 2, QBLK], F32, tag="S")
                    # even head on PE rows 0-63, odd head on rows 64-127:
                    # complementary row groups -> the two matmuls overlap
                    nc.tensor.matmul(
                        S[:, 0, :], lhsT=kt_t[0:HD, c * 128:(c + 1) * 128],
                        rhs=qt_t[0:HD, :], start=True, stop=True)
                    nc.tensor.matmul(
                        S[:, 1, :], lhsT=kt_t[HD:128, c * 128:(c + 1) * 128],
                        rhs=qt_t[HD:128, :], start=True, stop=True)
                    nc.scalar.activation(out=expT[:, c], in_=S[:, :, :], func=Exp)
                    if c >= 1:
                        attnv(c - 1)
                attnv(7)

                for par, ovx in ((0, ov0), (1, ov1)):
                    recip = norm_pool.tile([1, QBLK], F32, tag="recip")
                    nc.vector.reciprocal(out=recip, in_=ovx[HD:HD + 1, :])
                    bc_sb = norm_pool.tile([HD, QBLK], F32, tag="bc")
                    nc.gpsimd.partition_broadcast(bc_sb, recip, channels=HD)
                    if par == 0:
                        nc.vector.tensor_mul(
                            out=outT[0:HD, pj, :], in0=ovx[0:HD, :], in1=bc_sb)
                    else:
                        tmp = norm_pool.tile([HD, QBLK], F32R, tag="tmp")
                        nc.vector.tensor_mul(out=tmp, in0=ovx[0:HD, :], in1=bc_sb)
                        nc.sync.dma_start(outT[HD:128, pj, :], tmp)

            sb_wht = singles.tile([128, 8, D], F32R, tag="wht")
            for cc in range(8):
                nc.sync.dma_start(sb_wht[:, cc], wht.ap()[:, cc])

            # output projection: final[q, n] = outT.T @ WHT + bias
            for m in range(QBLK // 128):
                for jn in range(D // 512):
                    P = proj_ps.tile([128, 512], F32, tag="P")
                    for cc in range(8):
                        nc.tensor.matmul(
                            P,
                            lhsT=outT[:, cc, m * 128:(m + 1) * 128],
                            rhs=sb_wht[:, cc, jn * 512:(jn + 1) * 512],
                            start=(cc == 0), stop=(cc == 7))
                    Fo = final_pool.tile([128, 512], F32, tag="F")
                    nc.vector.tensor_add(
                        out=Fo, in0=P, in1=sb_bias[:, jn * 512:(jn + 1) * 512])
                    nc.sync.dma_start(
                        out.ap()[m * 128:(m + 1) * 128, jn * 512:(jn + 1) * 512],
                        Fo)

        if niter == 1:
            body()
        else:
            with tc.For_i(
                0, niter, 1,
                hint_engines=(
                    mybir.EngineType.PE,
                    mybir.EngineType.Activation,
                    mybir.EngineType.DVE,
                    mybir.EngineType.SP,
                    mybir.EngineType.Pool,
                ),
            ) as _i:
                body(_i)

    nc.compile()
    return nc


def _host_shard(Q, K, V, W_H, b_H):
    """Build the 8 per-core input dicts (all host-side numpy)."""
    Qs = (np.asarray(Q, np.float32) * SCALE)
    K = np.asarray(K, np.float32)
    V = np.asarray(V, np.float32)
    W_H = np.asarray(W_H, np.float32)
    b_H = np.asarray(b_H, np.float32)

    # [hd, n] chunked: [128, 8, D]
    wht = np.ascontiguousarray(W_H.T.reshape(8, 128, D).transpose(1, 0, 2))
    bias = np.ascontiguousarray(np.broadcast_to(b_H, (128, D))).astype(np.float32)

    in_maps = []
    for c in range(NCORES):
        b, half = divmod(c, 2)
        qlo = half * QBLK
        # [q, j, par, d] -> [par, d, j, q] -> [128, 8, QBLK]
        qtc = np.ascontiguousarray(
            Qs[b, qlo:qlo + QBLK].reshape(QBLK, 8, 2, HD).transpose(2, 3, 1, 0)
        ).reshape(128, 8, QBLK)
        ktc = np.ascontiguousarray(
            K[b].reshape(L, 8, 2, HD).transpose(2, 3, 1, 0)
        ).reshape(128, 8, L)
        # V_aug [k, h, 65] -> [c, p, h, e] -> [p, h, c, e]
        va = np.concatenate(
            [V[b].reshape(L, H, HD), np.ones((L, H, 1), np.float32)], axis=2)
        vvc = np.ascontiguousarray(
            va.reshape(8, 128, H, HD + 1).transpose(1, 2, 0, 3))
        in_maps.append({"qt": qtc, "kt": ktc, "vv": vvc, "wht": wht,
                        "bias": bias})
    return in_maps


def _get_runner(niter=1):
    """Build (once) and cache a jitted 8-core runner for the kernel."""
    key = ("runner", niter)
    if key in _STATE:
        return _STATE[key]

    import jax
    from jax.sharding import Mesh, PartitionSpec, NamedSharding
    from jax.experimental.shard_map import shard_map
    from concourse import bass2jax, mybir

    nc = _build_nc(niter)
    bass2jax.install_neuronx_cc_hook()

    partition_name = (
        nc.partition_id_tensor.name if nc.partition_id_tensor else None)
    in_names, out_names, out_avals, zero_shapes = [], [], [], []
    for alloc in nc.m.functions[0].allocations:
        if not isinstance(alloc, mybir.MemoryLocationSet):
            continue
        name = alloc.memorylocations[0].name
        if alloc.kind == "ExternalInput":
            if name != partition_name:
                in_names.append(name)
        elif alloc.kind == "ExternalOutput":
            out_names.append(name)
            shape = tuple(alloc.tensor_shape)
            dtype = mybir.dt.np(alloc.dtype)
            out_avals.append(jax.core.ShapedArray(shape, dtype))
            zero_shapes.append((shape, dtype))
    n_params = len(in_names)
    n_outs = len(out_avals)
    all_names = list(in_names) + list(out_names)
    if partition_name is not None:
        all_names.append(partition_name)
    donate = tuple(range(n_params, n_params + n_outs))

    def _body(*args):
        operands = list(args)
        if partition_name is not None:
            operands.append(bass2jax.partition_id_tensor())
        outs = bass2jax._bass_exec_p.bind(
            *operands,
            out_avals=tuple(out_avals),
            in_names=tuple(all_names),
            out_names=tuple(out_names),
            lowering_input_output_aliases=(),
            sim_require_finite=True,
            sim_require_nnan=True,
            nc=nc,
        )
        return tuple(outs)

    devices = jax.devices()[:NCORES]
    mesh = Mesh(np.asarray(devices), ("core",))
    in_specs = (PartitionSpec("core"),) * (n_params + n_outs)
    out_specs = (PartitionSpec("core"),) * n_outs
    sharded = jax.jit(
        shard_map(_body, mesh=mesh, in_specs=in_specs, out_specs=out_specs,
                  check_rep=False),
        donate_argnums=donate,
        keep_unused=True,
    )
    sharding = NamedSharding(mesh, PartitionSpec("core"))

    def put_inputs(in_maps):
        return [
            jax.device_put(
                np.concatenate(
                    [np.asarray(in_maps[c][nm]) for c in range(NCORES)], axis=0),
                sharding)
            for nm in in_names
        ]

    def run(in_maps, device_inputs=None):
        if device_inputs is None:
            device_inputs = put_inputs(in_maps)
        zeros = [
            jax.device_put(np.zeros((NCORES * s[0], *s[1:]), d), sharding)
            for s, d in zero_shapes
        ]
        out_arrs = sharded(*device_inputs, *zeros)
        results = []
        for c in range(NCORES):
            results.append({
                name: np.asarray(out_arrs[i]).reshape(
                    NCORES, *out_avals[i].shape)[c]
                for i, name in enumerate(out_names)
            })
        return results

    runner = {"run": run, "put_inputs": put_inputs, "sharded": sharded,
              "in_names": in_names, "out_names": out_names,
              "zero_shapes": zero_shapes, "nc": nc}
    _STATE[key] = runner
    return runner


def kernel(Q=None, K=None, V=None, W_H=None, b_H=None, mask=None, **kw):
    in_maps = _host_shard(Q, K, V, W_H, b_H)
    runner = _get_runner(niter=1)
    results = runner["run"](in_maps)
    out = np.empty((B, L, D), np.float32)
    for c in range(NCORES):
        b, half = divmod(c, 2)
        out[b, half * QBLK:(half + 1) * QBLK, :] = results[c]["out"]
    return out


# revision 10
# speedup vs baseline: 1.3601x; 1.3601x over previous
"""Multi-head attention + output projection, sharded over 8 TRN2 NeuronCores.

Problem: Q,K,V [4,1024,1024] f32; 16 heads x 64 dim; softmax(QK^T/sqrt(1024))V,
concat heads, out @ W_H.T + b_H.

Sharding: 8 cores = 4 batch x 2 query-halves. Each core computes full attention
(all 16 heads, all 1024 keys) for its 512 queries plus the output projection for
those rows. Output rows are disjoint -> no collectives.

Per-core kernel (all matmuls float32r: full PE rate, ~1.6e-4 matmul precision;
fp32 PSUM accumulation). Heads are processed in even/odd pairs whose QK^T
matmuls use complementary PE row groups (K=64 each at partition offsets 0/64)
so they can overlap in the array:
  scoresT[k,q] = KhT.T @ QhT   (contract d=64; Q pre-scaled by 1/sqrt(D) on host)
  expT = exp(scoresT)          (ACT, psum->sbuf, f32r; no max-subtraction:
                                |scores/32| <= ~1.5 for N(0,1) inputs)
  outT_aug[65,q] = sum_k V_aug[k,65].T @ expT[k,q]   (V has a ones column ->
                                row 64 = softmax denominator)
  normalize: DVE reciprocal + gpsimd partition_broadcast + DVE multiply
  final[q,n] = outT_norm.T @ W_H.T + b_H   (contract over 1024 = 8 chunks)
"""
import sys
import os

sys.path.insert(0, "/opt/trn_rl_repo")

import numpy as np

B, L, D, H, HD = 4, 1024, 1024, 16, 64
NCORES = 8
QBLK = L // 2  # 512 queries per core
SCALE = 1.0 / np.sqrt(np.float32(D))

_STATE = {}


def _build_nc(niter=1):
    import concourse.bass as bass
    import concourse.tile as tile
    from concourse import bacc, mybir
    from contextlib import ExitStack

    F32 = mybir.dt.float32
    F32R = mybir.dt.float32r
    Exp = mybir.ActivationFunctionType.Exp

    nc = bacc.Bacc("TRN2", target_bir_lowering=False, debug=False)
    qt = nc.dram_tensor("qt", [128, 8, QBLK], F32R, kind="ExternalInput")
    kt = nc.dram_tensor("kt", [128, 8, L], F32R, kind="ExternalInput")
    vv = nc.dram_tensor("vv", [128, H, 8, HD + 1], F32R, kind="ExternalInput")
    wht = nc.dram_tensor("wht", [128, 8, D], F32R, kind="ExternalInput")
    bias = nc.dram_tensor("bias", [128, D], F32, kind="ExternalInput")
    out = nc.dram_tensor("out", [QBLK, D], F32, kind="ExternalOutput")

    with tile.TileContext(nc) as tc, ExitStack() as ctx:
        singles = ctx.enter_context(tc.tile_pool(name="singles", bufs=1))
        qk_pool = ctx.enter_context(tc.tile_pool(name="qk", bufs=2))
        v_pool = ctx.enter_context(tc.tile_pool(name="vp", bufs=4))
        exp_pool = ctx.enter_context(tc.tile_pool(name="exp", bufs=2))
        norm_pool = ctx.enter_context(tc.tile_pool(name="norm", bufs=4))
        final_pool = ctx.enter_context(tc.tile_pool(name="final", bufs=2))
        scps = ctx.enter_context(tc.tile_pool(name="scps", bufs=2, space="PSUM"))
        ov_ps = ctx.enter_context(tc.tile_pool(name="ovps", bufs=2, space="PSUM"))
        proj_ps = ctx.enter_context(tc.tile_pool(name="prps", bufs=2, space="PSUM"))
        dram_pool = ctx.enter_context(tc.tile_pool(name="dram", bufs=2, space="DRAM"))

        def body(_=None):
            # warm the exp table while DMAs run
            warm_in = singles.tile([1, 8], F32, tag="warm_in")
            warm_out = singles.tile([1, 8], F32, tag="warm_out")
            nc.vector.memset(warm_in, 0.0)
            nc.scalar.activation(out=warm_out, in_=warm_in, func=Exp)

            sb_bias = singles.tile([128, D], F32, tag="bias")
            nc.sync.dma_start(sb_bias, bias.ap())

            # normalized concatenated attention output, transposed: [hd, q]
            outT = singles.tile([128, 8, QBLK], F32R, tag="outT")

            for pj in range(8):
                h0, h1 = 2 * pj, 2 * pj + 1
                qt_t = qk_pool.tile([128, QBLK], F32R, tag="qt")
                nc.sync.dma_start(qt_t, qt.ap()[:, pj])
                kt_t = qk_pool.tile([128, L], F32R, tag="kt")
                nc.sync.dma_start(kt_t, kt.ap()[:, pj])
                v0 = v_pool.tile([128, 8, HD + 1], F32R, tag="v")
                nc.sync.dma_start(v0, vv.ap()[:, h0])
                v1 = v_pool.tile([128, 8, HD + 1], F32R, tag="v")
                nc.sync.dma_start(v1, vv.ap()[:, h1])

                # expT[p, c, par, q]
                expT = exp_pool.tile([128, 8, 2, QBLK], F32R, tag="expT")
                ov0 = ov_ps.tile([HD + 1, QBLK], F32, tag="ov")
                ov1 = ov_ps.tile([HD + 1, QBLK], F32, tag="ov")

                def attnv(c):
                    nc.tensor.matmul(
                        ov0[:, :], lhsT=v0[:, c, :], rhs=expT[:, c, 0, :],
                        start=(c == 0), stop=(c == 7))
                    nc.tensor.matmul(
                        ov1[:, :], lhsT=v1[:, c, :], rhs=expT[:, c, 1, :],
                        start=(c == 0), stop=(c == 7))

                for c in range(8):
                    S = scps.tile([128, 2, QBLK], F32, tag="S")
                    # even head on PE rows 0-63, odd head on rows 64-127:
                    # complementary row groups -> the two matmuls overlap
                    nc.tensor.matmul(
                        S[:, 0, :], lhsT=kt_t[0:HD, c * 128:(c + 1) * 128],
                        rhs=qt_t[0:HD, :], start=True, stop=True)
                    nc.tensor.matmul(
                        S[:, 1, :], lhsT=kt_t[HD:128, c * 128:(c + 1) * 128],
                        rhs=qt_t[HD:128, :], start=True, stop=True)
                    nc.scalar.activation(out=expT[:, c], in_=S[:, :, :], func=Exp)
                    if c >= 1:
                        attnv(c - 1)
                attnv(7)

                for par, ovx in ((0, ov0), (1, ov1)):
                    # evacuate the psum accumulator right away so the ov slot
                    # frees for the next pair; normalize from the sbuf copy
                    ovs = norm_pool.tile([HD + 1, QBLK], F32, tag="ovs")
                    nc.vector.tensor_copy(out=ovs, in_=ovx)
                    recip = norm_pool.tile([1, QBLK], F32, tag="recip")
                    nc.vector.reciprocal(out=recip, in_=ovs[HD:HD + 1, :])
                    # broadcast recip across 64 partitions via a DRAM roundtrip
                    dsc = dram_pool.tile([1, QBLK], F32, tag="dsc")
                    nc.sync.dma_start(dsc, recip)
                    bc_sb = norm_pool.tile([HD, QBLK], F32, tag="bc")
                    nc.sync.dma_start(bc_sb, dsc[0:1, :].partition_broadcast(HD))
                    if par == 0:
                        nc.vector.tensor_mul(
                            out=outT[0:HD, pj, :], in0=ovs[0:HD, :], in1=bc_sb)
                    else:
                        tmp = norm_pool.tile([HD, QBLK], F32R, tag="tmp")
                        nc.vector.tensor_mul(out=tmp, in0=ovs[0:HD, :], in1=bc_sb)
                        nc.sync.dma_start(outT[HD:128, pj, :], tmp)

            sb_wht = singles.tile([128, 8, D], F32R, tag="wht")
            for cc in range(8):
                nc.sync.dma_start(sb_wht[:, cc], wht.ap()[:, cc])

            # output projection: final[q, n] = outT.T @ WHT + bias
            for m in range(QBLK // 128):
                for jn in range(D // 512):
                    P = proj_ps.tile([128, 512], F32, tag="P")
                    for cc in range(8):
                        nc.tensor.matmul(
                            P,
                            lhsT=outT[:, cc, m * 128:(m + 1) * 128],
                            rhs=sb_wht[:, cc, jn * 512:(jn + 1) * 512],
                            start=(cc == 0), stop=(cc == 7))
                    Fo = final_pool.tile([128, 512], F32, tag="F")
                    nc.vector.tensor_add(
                        out=Fo, in0=P, in1=sb_bias[:, jn * 512:(jn + 1) * 512])
                    nc.sync.dma_start(
                        out.ap()[m * 128:(m + 1) * 128, jn * 512:(jn + 1) * 512],
                        Fo)

        if niter == 1:
            body()
        else:
            with tc.For_i(
                0, niter, 1,
                hint_engines=(
                    mybir.EngineType.PE,
                    mybir.EngineType.Activation,
                    mybir.EngineType.DVE,
                    mybir.EngineType.SP,
                    mybir.EngineType.Pool,
                ),
            ) as _i:
                body(_i)

    nc.compile()
    return nc


def _host_shard(Q, K, V, W_H, b_H):
    """Build the 8 per-core input dicts (all host-side numpy)."""
    Qs = (np.asarray(Q, np.float32) * SCALE)
    K = np.asarray(K, np.float32)
    V = np.asarray(V, np.float32)
    W_H = np.asarray(W_H, np.float32)
    b_H = np.asarray(b_H, np.float32)

    # [hd, n] chunked: [128, 8, D]
    wht = np.ascontiguousarray(W_H.T.reshape(8, 128, D).transpose(1, 0, 2))
    bias = np.ascontiguousarray(np.broadcast_to(b_H, (128, D))).astype(np.float32)

    in_maps = []
    for c in range(NCORES):
        b, half = divmod(c, 2)
        qlo = half * QBLK
        # [q, j, par, d] -> [par, d, j, q] -> [128, 8, QBLK]
        qtc = np.ascontiguousarray(
            Qs[b, qlo:qlo + QBLK].reshape(QBLK, 8, 2, HD).transpose(2, 3, 1, 0)
        ).reshape(128, 8, QBLK)
        ktc = np.ascontiguousarray(
            K[b].reshape(L, 8, 2, HD).transpose(2, 3, 1, 0)
        ).reshape(128, 8, L)
        # V_aug [k, h, 65] -> [c, p, h, e] -> [p, h, c, e]
        va = np.concatenate(
            [V[b].reshape(L, H, HD), np.ones((L, H, 1), np.float32)], axis=2)
        vvc = np.ascontiguousarray(
            va.reshape(8, 128, H, HD + 1).transpose(1, 2, 0, 3))
        in_maps.append({"qt": qtc, "kt": ktc, "vv": vvc, "wht": wht,
                        "bias": bias})
    return in_maps


def _get_runner(niter=1):
    """Build (once) and cache a jitted 8-core runner for the kernel."""
    key = ("runner", niter)
    if key in _STATE:
        return _STATE[key]

    import jax
    from jax.sharding import Mesh, PartitionSpec, NamedSharding
    from jax.experimental.shard_map import shard_map
    from concourse import bass2jax, mybir

    nc = _build_nc(niter)
    bass2jax.install_neuronx_cc_hook()

    partition_name = (
        nc.partition_id_tensor.name if nc.partition_id_tensor else None)
    in_names, out_names, out_avals, zero_shapes = [], [], [], []
    for alloc in nc.m.functions[0].allocations:
        if not isinstance(alloc, mybir.MemoryLocationSet):
            continue
        name = alloc.memorylocations[0].name
        if alloc.kind == "ExternalInput":
            if name != partition_name:
                in_names.append(name)
        elif alloc.kind == "ExternalOutput":
            out_names.append(name)
            shape = tuple(alloc.tensor_shape)
            dtype = mybir.dt.np(alloc.dtype)
            out_avals.append(jax.core.ShapedArray(shape, dtype))
            zero_shapes.append((shape, dtype))
    n_params = len(in_names)
    n_outs = len(out_avals)
    all_names = list(in_names) + list(out_names)
    if partition_name is not None:
        all_names.append(partition_name)
    donate = tuple(range(n_params, n_params + n_outs))

    def _body(*args):
        operands = list(args)
        if partition_name is not None:
            operands.append(bass2jax.partition_id_tensor())
        outs = bass2jax._bass_exec_p.bind(
            *operands,
            out_avals=tuple(out_avals),
            in_names=tuple(all_names),
            out_names=tuple(out_names),
            lowering_input_output_aliases=(),
            sim_require_finite=True,
            sim_require_nnan=True,
            nc=nc,
        )
        return tuple(outs)

    devices = jax.devices()[:NCORES]
    mesh = Mesh(np.asarray(devices), ("core",))
    in_specs = (PartitionSpec("core"),) * (n_params + n_outs)
    out_specs = (PartitionSpec("core"),) * n_outs
    sharded = jax.jit(
        shard_map(_body, mesh=mesh, in_specs=in_specs, out_specs=out_specs,
                  check_rep=False),
        donate_argnums=donate,
        keep_unused=True,
    )
    sharding = NamedSharding(mesh, PartitionSpec("core"))

    def put_inputs(in_maps):
        return [
            jax.device_put(
                np.concatenate(
                    [np.asarray(in_maps[c][nm]) for c in range(NCORES)], axis=0),
                sharding)
            for nm in in_names
        ]

    def run(in_maps, device_inputs=None):
        if device_inputs is None:
            device_inputs = put_inputs(in_maps)
        zeros = [
            jax.device_put(np.zeros((NCORES * s[0], *s[1:]), d), sharding)
            for s, d in zero_shapes
        ]
        out_arrs = sharded(*device_inputs, *zeros)
        results = []
        for c in range(NCORES):
            results.append({
                name: np.asarray(out_arrs[i]).reshape(
                    NCORES, *out_avals[i].shape)[c]
                for i, name in enumerate(out_names)
            })
        return results

    runner = {"run": run, "put_inputs": put_inputs, "sharded": sharded,
              "in_names": in_names, "out_names": out_names,
              "zero_shapes": zero_shapes, "nc": nc}
    _STATE[key] = runner
    return runner


def kernel(Q=None, K=None, V=None, W_H=None, b_H=None, mask=None, **kw):
    in_maps = _host_shard(Q, K, V, W_H, b_H)
    runner = _get_runner(niter=1)
    results = runner["run"](in_maps)
    out = np.empty((B, L, D), np.float32)
    for c in range(NCORES):
        b, half = divmod(c, 2)
        out[b, half * QBLK:(half + 1) * QBLK, :] = results[c]["out"]
    return out


# revision 11
# speedup vs baseline: 2.1673x; 1.5935x over previous
"""Multi-head attention + output projection, sharded over 8 TRN2 NeuronCores.

Problem: Q,K,V [4,1024,1024] f32; 16 heads x 64 dim; softmax(QK^T/sqrt(1024))V,
concat heads, out @ W_H.T + b_H.

Sharding: 8 cores = 4 batch x 2 query-halves. Each core computes full attention
(all 16 heads, all 1024 keys) for its 512 queries plus the output projection for
those rows. Output rows are disjoint -> no collectives.

Per-core kernel (all matmuls float32r: full PE rate, ~1.6e-4 matmul precision;
fp32 PSUM accumulation). Heads are processed in even/odd pairs whose QK^T
matmuls use complementary PE row groups (K=64 each at partition offsets 0/64)
so they can overlap in the array:
  scoresT[k,q] = KhT.T @ QhT   (contract d=64; Q pre-scaled by 1/sqrt(D) on host)
  expT = exp(scoresT)          (ACT, psum->sbuf, f32r; no max-subtraction:
                                |scores/32| <= ~1.5 for N(0,1) inputs)
  outT_aug[65,q] = sum_k V_aug[k,65].T @ expT[k,q]   (V has a ones column ->
                                row 64 = softmax denominator)
  normalize: DVE reciprocal + gpsimd partition_broadcast + DVE multiply
  final[q,n] = outT_norm.T @ W_H.T + b_H   (contract over 1024 = 8 chunks)
"""
import sys
import os

sys.path.insert(0, "/opt/trn_rl_repo")

import numpy as np

B, L, D, H, HD = 4, 1024, 1024, 16, 64
NCORES = 8
QBLK = L // 2  # 512 queries per core
SCALE = 1.0 / np.sqrt(np.float32(D))

_STATE = {}


def _build_nc(niter=1):
    import concourse.bass as bass
    import concourse.tile as tile
    from concourse import bacc, mybir
    from contextlib import ExitStack

    F32 = mybir.dt.float32
    F32R = mybir.dt.float32r
    Exp = mybir.ActivationFunctionType.Exp

    nc = bacc.Bacc("TRN2", target_bir_lowering=False, debug=False, use_seq_codegen=True)
    qt = nc.dram_tensor("qt", [128, 8, QBLK], F32R, kind="ExternalInput")
    kt = nc.dram_tensor("kt", [128, 8, L], F32R, kind="ExternalInput")
    vv = nc.dram_tensor("vv", [128, H, 8, HD + 1], F32R, kind="ExternalInput")
    wht = nc.dram_tensor("wht", [128, 8, D], F32R, kind="ExternalInput")
    bias = nc.dram_tensor("bias", [128, D], F32, kind="ExternalInput")
    out = nc.dram_tensor("out", [QBLK, D], F32, kind="ExternalOutput")

    with tile.TileContext(nc) as tc, ExitStack() as ctx:
        singles = ctx.enter_context(tc.tile_pool(name="singles", bufs=1))
        qk_pool = ctx.enter_context(tc.tile_pool(name="qk", bufs=2))
        v_pool = ctx.enter_context(tc.tile_pool(name="vp", bufs=4))
        exp_pool = ctx.enter_context(tc.tile_pool(name="exp", bufs=2))
        norm_pool = ctx.enter_context(tc.tile_pool(name="norm", bufs=4))
        final_pool = ctx.enter_context(tc.tile_pool(name="final", bufs=2))
        scps = ctx.enter_context(tc.tile_pool(name="scps", bufs=2, space="PSUM"))
        ov_ps = ctx.enter_context(tc.tile_pool(name="ovps", bufs=2, space="PSUM"))
        proj_ps = ctx.enter_context(tc.tile_pool(name="prps", bufs=2, space="PSUM"))
        dram_pool = ctx.enter_context(tc.tile_pool(name="dram", bufs=2, space="DRAM"))

        def body(_=None):
            # warm the exp table while DMAs run
            warm_in = singles.tile([1, 8], F32, tag="warm_in")
            warm_out = singles.tile([1, 8], F32, tag="warm_out")
            nc.vector.memset(warm_in, 0.0)
            nc.scalar.activation(out=warm_out, in_=warm_in, func=Exp)

            sb_bias = singles.tile([128, D], F32, tag="bias")
            nc.sync.dma_start(sb_bias, bias.ap())

            # normalized concatenated attention output, transposed: [hd, q]
            outT = singles.tile([128, 8, QBLK], F32R, tag="outT")

            for pj in range(8):
                h0, h1 = 2 * pj, 2 * pj + 1
                qt_t = qk_pool.tile([128, QBLK], F32R, tag="qt")
                nc.sync.dma_start(qt_t, qt.ap()[:, pj])
                kt_t = qk_pool.tile([128, L], F32R, tag="kt")
                nc.sync.dma_start(kt_t, kt.ap()[:, pj])
                v0 = v_pool.tile([128, 8, HD + 1], F32R, tag="v")
                nc.sync.dma_start(v0, vv.ap()[:, h0])
                v1 = v_pool.tile([128, 8, HD + 1], F32R, tag="v")
                nc.sync.dma_start(v1, vv.ap()[:, h1])

                # expT[p, c, par, q]
                expT = exp_pool.tile([128, 8, 2, QBLK], F32R, tag="expT")
                ov0 = ov_ps.tile([HD + 1, QBLK], F32, tag="ov")
                ov1 = ov_ps.tile([HD + 1, QBLK], F32, tag="ov")

                def attnv(c):
                    nc.tensor.matmul(
                        ov0[:, :], lhsT=v0[:, c, :], rhs=expT[:, c, 0, :],
                        start=(c == 0), stop=(c == 7))
                    nc.tensor.matmul(
                        ov1[:, :], lhsT=v1[:, c, :], rhs=expT[:, c, 1, :],
                        start=(c == 0), stop=(c == 7))

                for c in range(8):
                    S = scps.tile([128, 2, QBLK], F32, tag="S")
                    # even head on PE rows 0-63, odd head on rows 64-127:
                    # complementary row groups -> the two matmuls overlap
                    nc.tensor.matmul(
                        S[:, 0, :], lhsT=kt_t[0:HD, c * 128:(c + 1) * 128],
                        rhs=qt_t[0:HD, :], start=True, stop=True)
                    nc.tensor.matmul(
                        S[:, 1, :], lhsT=kt_t[HD:128, c * 128:(c + 1) * 128],
                        rhs=qt_t[HD:128, :], start=True, stop=True)
                    nc.scalar.activation(out=expT[:, c], in_=S[:, :, :], func=Exp)
                    if c >= 1:
                        attnv(c - 1)
                attnv(7)

                for par, ovx in ((0, ov0), (1, ov1)):
                    # evacuate the psum accumulator right away so the ov slot
                    # frees for the next pair; normalize from the sbuf copy
                    ovs = norm_pool.tile([HD + 1, QBLK], F32, tag="ovs")
                    nc.vector.tensor_copy(out=ovs, in_=ovx)
                    recip = norm_pool.tile([1, QBLK], F32, tag="recip")
                    nc.vector.reciprocal(out=recip, in_=ovs[HD:HD + 1, :])
                    # broadcast recip across 64 partitions via a DRAM roundtrip
                    dsc = dram_pool.tile([1, QBLK], F32, tag="dsc")
                    nc.sync.dma_start(dsc, recip)
                    bc_sb = norm_pool.tile([HD, QBLK], F32, tag="bc")
                    nc.sync.dma_start(bc_sb, dsc[0:1, :].partition_broadcast(HD))
                    if par == 0:
                        nc.vector.tensor_mul(
                            out=outT[0:HD, pj, :], in0=ovs[0:HD, :], in1=bc_sb)
                    else:
                        tmp = norm_pool.tile([HD, QBLK], F32R, tag="tmp")
                        nc.vector.tensor_mul(out=tmp, in0=ovs[0:HD, :], in1=bc_sb)
                        nc.sync.dma_start(outT[HD:128, pj, :], tmp)

            sb_wht = singles.tile([128, 8, D], F32R, tag="wht")
            for cc in range(8):
                nc.sync.dma_start(sb_wht[:, cc], wht.ap()[:, cc])

            # output projection: final[q, n] = outT.T @ WHT + bias
            for m in range(QBLK // 128):
                for jn in range(D // 512):
                    P = proj_ps.tile([128, 512], F32, tag="P")
                    for cc in range(8):
                        nc.tensor.matmul(
                            P,
                            lhsT=outT[:, cc, m * 128:(m + 1) * 128],
                            rhs=sb_wht[:, cc, jn * 512:(jn + 1) * 512],
                            start=(cc == 0), stop=(cc == 7))
                    Fo = final_pool.tile([128, 512], F32, tag="F")
                    nc.vector.tensor_add(
                        out=Fo, in0=P, in1=sb_bias[:, jn * 512:(jn + 1) * 512])
                    nc.sync.dma_start(
                        out.ap()[m * 128:(m + 1) * 128, jn * 512:(jn + 1) * 512],
                        Fo)

        if niter == 1:
            body()
        else:
            with tc.For_i(
                0, niter, 1,
                hint_engines=(
                    mybir.EngineType.PE,
                    mybir.EngineType.Activation,
                    mybir.EngineType.DVE,
                    mybir.EngineType.SP,
                    mybir.EngineType.Pool,
                ),
            ) as _i:
                body(_i)

    nc.compile()
    return nc


def _host_shard(Q, K, V, W_H, b_H):
    """Build the 8 per-core input dicts (all host-side numpy)."""
    Qs = (np.asarray(Q, np.float32) * SCALE)
    K = np.asarray(K, np.float32)
    V = np.asarray(V, np.float32)
    W_H = np.asarray(W_H, np.float32)
    b_H = np.asarray(b_H, np.float32)

    # [hd, n] chunked: [128, 8, D]
    wht = np.ascontiguousarray(W_H.T.reshape(8, 128, D).transpose(1, 0, 2))
    bias = np.ascontiguousarray(np.broadcast_to(b_H, (128, D))).astype(np.float32)

    in_maps = []
    for c in range(NCORES):
        b, half = divmod(c, 2)
        qlo = half * QBLK
        # [q, j, par, d] -> [par, d, j, q] -> [128, 8, QBLK]
        qtc = np.ascontiguousarray(
            Qs[b, qlo:qlo + QBLK].reshape(QBLK, 8, 2, HD).transpose(2, 3, 1, 0)
        ).reshape(128, 8, QBLK)
        ktc = np.ascontiguousarray(
            K[b].reshape(L, 8, 2, HD).transpose(2, 3, 1, 0)
        ).reshape(128, 8, L)
        # V_aug [k, h, 65] -> [c, p, h, e] -> [p, h, c, e]
        va = np.concatenate(
            [V[b].reshape(L, H, HD), np.ones((L, H, 1), np.float32)], axis=2)
        vvc = np.ascontiguousarray(
            va.reshape(8, 128, H, HD + 1).transpose(1, 2, 0, 3))
        in_maps.append({"qt": qtc, "kt": ktc, "vv": vvc, "wht": wht,
                        "bias": bias})
    return in_maps


def _get_runner(niter=1):
    """Build (once) and cache a jitted 8-core runner for the kernel."""
    key = ("runner", niter)
    if key in _STATE:
        return _STATE[key]

    import jax
    from jax.sharding import Mesh, PartitionSpec, NamedSharding
    from jax.experimental.shard_map import shard_map
    from concourse import bass2jax, mybir

    nc = _build_nc(niter)
    bass2jax.install_neuronx_cc_hook()

    partition_name = (
        nc.partition_id_tensor.name if nc.partition_id_tensor else None)
    in_names, out_names, out_avals, zero_shapes = [], [], [], []
    for alloc in nc.m.functions[0].allocations:
        if not isinstance(alloc, mybir.MemoryLocationSet):
            continue
        name = alloc.memorylocations[0].name
        if alloc.kind == "ExternalInput":
            if name != partition_name:
                in_names.append(name)
        elif alloc.kind == "ExternalOutput":
            out_names.append(name)
            shape = tuple(alloc.tensor_shape)
            dtype = mybir.dt.np(alloc.dtype)
            out_avals.append(jax.core.ShapedArray(shape, dtype))
            zero_shapes.append((shape, dtype))
    n_params = len(in_names)
    n_outs = len(out_avals)
    all_names = list(in_names) + list(out_names)
    if partition_name is not None:
        all_names.append(partition_name)
    donate = tuple(range(n_params, n_params + n_outs))

    def _body(*args):
        operands = list(args)
        if partition_name is not None:
            operands.append(bass2jax.partition_id_tensor())
        outs = bass2jax._bass_exec_p.bind(
            *operands,
            out_avals=tuple(out_avals),
            in_names=tuple(all_names),
            out_names=tuple(out_names),
            lowering_input_output_aliases=(),
            sim_require_finite=True,
            sim_require_nnan=True,
            nc=nc,
        )
        return tuple(outs)

    devices = jax.devices()[:NCORES]
    mesh = Mesh(np.asarray(devices), ("core",))
    in_specs = (PartitionSpec("core"),) * (n_params + n_outs)
    out_specs = (PartitionSpec("core"),) * n_outs
    sharded = jax.jit(
        shard_map(_body, mesh=mesh, in_specs=in_specs, out_specs=out_specs,
                  check_rep=False),
        donate_argnums=donate,
        keep_unused=True,
    )
    sharding = NamedSharding(mesh, PartitionSpec("core"))

    def put_inputs(in_maps):
        return [
            jax.device_put(
                np.concatenate(
                    [np.asarray(in_maps[c][nm]) for c in range(NCORES)], axis=0),
                sharding)
            for nm in in_names
        ]

    def run(in_maps, device_inputs=None):
        if device_inputs is None:
            device_inputs = put_inputs(in_maps)
        zeros = [
            jax.device_put(np.zeros((NCORES * s[0], *s[1:]), d), sharding)
            for s, d in zero_shapes
        ]
        out_arrs = sharded(*device_inputs, *zeros)
        results = []
        for c in range(NCORES):
            results.append({
                name: np.asarray(out_arrs[i]).reshape(
                    NCORES, *out_avals[i].shape)[c]
                for i, name in enumerate(out_names)
            })
        return results

    runner = {"run": run, "put_inputs": put_inputs, "sharded": sharded,
              "in_names": in_names, "out_names": out_names,
              "zero_shapes": zero_shapes, "nc": nc}
    _STATE[key] = runner
    return runner


def kernel(Q=None, K=None, V=None, W_H=None, b_H=None, mask=None, **kw):
    in_maps = _host_shard(Q, K, V, W_H, b_H)
    runner = _get_runner(niter=1)
    results = runner["run"](in_maps)
    out = np.empty((B, L, D), np.float32)
    for c in range(NCORES):
        b, half = divmod(c, 2)
        out[b, half * QBLK:(half + 1) * QBLK, :] = results[c]["out"]
    return out


# revision 15
# speedup vs baseline: 5.7828x; 2.6682x over previous
"""Multi-head attention + output projection, sharded over 8 TRN2 NeuronCores.

Problem: Q,K,V [4,1024,1024] f32; 16 heads x 64 dim; softmax(QK^T/sqrt(1024))V,
concat heads, out @ W_H.T + b_H.

Sharding: 8 cores = 4 batch x 2 query-halves. Each core computes full attention
(all 16 heads, all 1024 keys) for its 512 queries plus the output projection for
those rows. Output rows are disjoint -> no collectives.

Per-core kernel (all matmuls float32r: full PE rate, ~1.6e-4 matmul precision;
fp32 PSUM accumulation). Heads are processed in even/odd pairs whose QK^T
matmuls use complementary PE row groups (K=64 each at partition offsets 0/64)
so they can overlap in the array:
  scoresT[k,q] = KhT.T @ QhT   (contract d=64; Q pre-scaled by 1/sqrt(D) on host)
  expT = exp(scoresT)          (ACT, psum->sbuf, f32r; no max-subtraction:
                                |scores/32| <= ~1.5 for N(0,1) inputs)
  outT_aug[65,q] = sum_k V_aug[k,65].T @ expT[k,q]   (V has a ones column ->
                                row 64 = softmax denominator)
  normalize: DVE reciprocal + gpsimd partition_broadcast + DVE multiply
  final[q,n] = outT_norm.T @ W_H.T + b_H   (contract over 1024 = 8 chunks)
"""
import sys
import os

sys.path.insert(0, "/opt/trn_rl_repo")

import numpy as np

B, L, D, H, HD = 4, 1024, 1024, 16, 64
NCORES = 8
QBLK = L // 2  # 512 queries per core
SCALE = 1.0 / np.sqrt(np.float32(D))

_STATE = {}


def _build_nc(niter=1, ablate="full"):
    import concourse.bass as bass
    import concourse.tile as tile
    from concourse import bacc, mybir
    from contextlib import ExitStack

    F32 = mybir.dt.float32
    F32R = mybir.dt.float32r
    Exp = mybir.ActivationFunctionType.Exp

    nc = bacc.Bacc("TRN2", target_bir_lowering=False, debug=False, use_seq_codegen=True)
    qt = nc.dram_tensor("qt", [128, 8, QBLK], F32R, kind="ExternalInput")
    kt = nc.dram_tensor("kt", [128, 8, L], F32R, kind="ExternalInput")
    vv = nc.dram_tensor("vv", [128, H, 8, HD + 1], F32R, kind="ExternalInput")
    wht = nc.dram_tensor("wht", [128, 8, D], F32R, kind="ExternalInput")
    bias = nc.dram_tensor("bias", [128, D], F32, kind="ExternalInput")
    out = nc.dram_tensor("out", [QBLK, D], F32, kind="ExternalOutput")

    with tile.TileContext(nc) as tc, ExitStack() as ctx:
        singles = ctx.enter_context(tc.tile_pool(name="singles", bufs=1))
        qk_pool = ctx.enter_context(tc.tile_pool(name="qk", bufs=2))
        v_pool = ctx.enter_context(tc.tile_pool(name="vp", bufs=4))
        exp_pool = ctx.enter_context(tc.tile_pool(name="exp", bufs=2))
        norm_pool = ctx.enter_context(tc.tile_pool(name="norm", bufs=4))
        final_pool = ctx.enter_context(tc.tile_pool(name="final", bufs=2))
        scps = ctx.enter_context(tc.tile_pool(name="scps", bufs=2, space="PSUM"))
        ov_ps = ctx.enter_context(tc.tile_pool(name="ovps", bufs=2, space="PSUM"))
        proj_ps = ctx.enter_context(tc.tile_pool(name="prps", bufs=2, space="PSUM"))
        dram_pool = ctx.enter_context(tc.tile_pool(name="dram", bufs=2, space="DRAM"))

        def body(_=None):
            # warm the exp table while DMAs run
            warm_in = singles.tile([1, 8], F32, tag="warm_in")
            warm_out = singles.tile([1, 8], F32, tag="warm_out")
            nc.vector.memset(warm_in, 0.0)
            nc.scalar.activation(out=warm_out, in_=warm_in, func=Exp)

            sb_bias = singles.tile([128, D], F32, tag="bias")
            nc.sync.dma_start(sb_bias, bias.ap())

            # normalized concatenated attention output, transposed: [hd, q]
            outT = singles.tile([128, 8, QBLK], F32R, tag="outT")

            for pj in range(8):
                h0, h1 = 2 * pj, 2 * pj + 1
                qt_t = qk_pool.tile([128, QBLK], F32R, tag="qt")
                nc.sync.dma_start(qt_t, qt.ap()[:, pj])
                kt_t = qk_pool.tile([128, L], F32R, tag="kt")
                nc.sync.dma_start(kt_t, kt.ap()[:, pj])
                if ablate != "scores":
                    v0 = v_pool.tile([128, 8, HD + 1], F32R, tag="v")
                    nc.sync.dma_start(v0, vv.ap()[:, h0])
                    v1 = v_pool.tile([128, 8, HD + 1], F32R, tag="v")
                    nc.sync.dma_start(v1, vv.ap()[:, h1])

                # expT[p, c, par, q]
                expT = exp_pool.tile([128, 8, 2, QBLK], F32R, tag="expT")
                if ablate != "scores":
                    ov0 = ov_ps.tile([HD + 1, QBLK], F32, tag="ov")
                    ov1 = ov_ps.tile([HD + 1, QBLK], F32, tag="ov")

                def attnv(c):
                    nc.tensor.matmul(
                        ov0[:, :], lhsT=v0[:, c, :], rhs=expT[:, c, 0, :],
                        start=(c == 0), stop=(c == 7))
                    nc.tensor.matmul(
                        ov1[:, :], lhsT=v1[:, c, :], rhs=expT[:, c, 1, :],
                        start=(c == 0), stop=(c == 7))

                for c in range(8):
                    S = scps.tile([128, 2, QBLK], F32, tag="S")
                    # even head on PE rows 0-63, odd head on rows 64-127:
                    # complementary row groups -> the two matmuls overlap
                    nc.tensor.matmul(
                        S[:, 0, :], lhsT=kt_t[0:HD, c * 128:(c + 1) * 128],
                        rhs=qt_t[0:HD, :], start=True, stop=True)
                    nc.tensor.matmul(
                        S[:, 1, :], lhsT=kt_t[HD:128, c * 128:(c + 1) * 128],
                        rhs=qt_t[HD:128, :], start=True, stop=True)
                    nc.scalar.activation(out=expT[:, c], in_=S[:, :, :], func=Exp)
                    if ablate != "scores" and c >= 1:
                        attnv(c - 1)
                if ablate != "scores":
                    attnv(7)
                else:
                    # keep expT live so nothing is dead-code eliminated
                    nc.vector.tensor_copy(out=outT[:, pj, 0:2], in_=expT[:, 7, 0, 0:2])
                    continue
                for par, ovx in ((0, ov0), (1, ov1)):
                    # evacuate the psum accumulator right away so the ov slot
                    # frees for the next pair; normalize from the sbuf copy
                    ovs = norm_pool.tile([HD + 1, QBLK], F32, tag="ovs")
                    nc.vector.tensor_copy(out=ovs, in_=ovx)
                    recip = norm_pool.tile([1, QBLK], F32, tag="recip")
                    nc.vector.reciprocal(out=recip, in_=ovs[HD:HD + 1, :])
                    # broadcast recip across 64 partitions via a DRAM roundtrip
                    dsc = dram_pool.tile([1, QBLK], F32, tag="dsc")
                    nc.gpsimd.dma_start(dsc, recip)
                    bc_sb = norm_pool.tile([HD, QBLK], F32, tag="bc")
                    nc.gpsimd.dma_start(bc_sb, dsc[0:1, :].partition_broadcast(HD))
                    if par == 0:
                        nc.vector.tensor_mul(
                            out=outT[0:HD, pj, :], in0=ovs[0:HD, :], in1=bc_sb)
                    else:
                        tmp = norm_pool.tile([HD, QBLK], F32R, tag="tmp")
                        nc.vector.tensor_mul(out=tmp, in0=ovs[0:HD, :], in1=bc_sb)
                        nc.gpsimd.dma_start(outT[HD:128, pj, :], tmp)

            sb_wht = singles.tile([128, 8, D], F32R, tag="wht")
            for cc in range(8):
                nc.sync.dma_start(sb_wht[:, cc], wht.ap()[:, cc])

            # output projection: final[q, n] = outT.T @ WHT + bias
            if ablate in ("noproj", "scores"):
                Fo = final_pool.tile([128, 512], F32, tag="F")
                nc.vector.tensor_copy(out=Fo, in_=outT[:, 0, :].bitcast(F32))
                nc.sync.dma_start(out.ap()[0:128, 0:512], Fo)
            proj_ms = [] if ablate in ("scores", "noproj") else list(range(QBLK // 128))
            for m in proj_ms:
                for jn in range(D // 512):
                    P = proj_ps.tile([128, 512], F32, tag="P")
                    for cc in range(8):
                        nc.tensor.matmul(
                            P,
                            lhsT=outT[:, cc, m * 128:(m + 1) * 128],
                            rhs=sb_wht[:, cc, jn * 512:(jn + 1) * 512],
                            start=(cc == 0), stop=(cc == 7))
                    Fo = final_pool.tile([128, 512], F32, tag="F")
                    nc.vector.tensor_add(
                        out=Fo, in0=P, in1=sb_bias[:, jn * 512:(jn + 1) * 512])
                    nc.sync.dma_start(
                        out.ap()[m * 128:(m + 1) * 128, jn * 512:(jn + 1) * 512],
                        Fo)

        if niter == 1:
            body()
        else:
            with tc.For_i(
                0, niter, 1,
                hint_engines=(
                    mybir.EngineType.PE,
                    mybir.EngineType.Activation,
                    mybir.EngineType.DVE,
                    mybir.EngineType.SP,
                    mybir.EngineType.Pool,
                ),
            ) as _i:
                body(_i)

    nc.compile()
    return nc


def _host_shard(Q, K, V, W_H, b_H):
    """Build the 8 per-core input dicts (all host-side numpy)."""
    Qs = (np.asarray(Q, np.float32) * SCALE)
    K = np.asarray(K, np.float32)
    V = np.asarray(V, np.float32)
    W_H = np.asarray(W_H, np.float32)
    b_H = np.asarray(b_H, np.float32)

    # [hd, n] chunked: [128, 8, D]
    wht = np.ascontiguousarray(W_H.T.reshape(8, 128, D).transpose(1, 0, 2))
    bias = np.ascontiguousarray(np.broadcast_to(b_H, (128, D))).astype(np.float32)

    in_maps = []
    for c in range(NCORES):
        b, half = divmod(c, 2)
        qlo = half * QBLK
        # [q, j, par, d] -> [par, d, j, q] -> [128, 8, QBLK]
        qtc = np.ascontiguousarray(
            Qs[b, qlo:qlo + QBLK].reshape(QBLK, 8, 2, HD).transpose(2, 3, 1, 0)
        ).reshape(128, 8, QBLK)
        ktc = np.ascontiguousarray(
            K[b].reshape(L, 8, 2, HD).transpose(2, 3, 1, 0)
        ).reshape(128, 8, L)
        # V_aug [k, h, 65] -> [c, p, h, e] -> [p, h, c, e]
        va = np.concatenate(
            [V[b].reshape(L, H, HD), np.ones((L, H, 1), np.float32)], axis=2)
        vvc = np.ascontiguousarray(
            va.reshape(8, 128, H, HD + 1).transpose(1, 2, 0, 3))
        in_maps.append({"qt": qtc, "kt": ktc, "vv": vvc, "wht": wht,
                        "bias": bias})
    return in_maps


def _get_runner(niter=1):
    """Build (once) and cache a jitted 8-core runner for the kernel."""
    import os as _os
    ablate = _os.environ.get("KABLATE", "full")
    key = ("runner", niter, ablate)
    if key in _STATE:
        return _STATE[key]

    import jax
    from jax.sharding import Mesh, PartitionSpec, NamedSharding
    from jax.experimental.shard_map import shard_map
    from concourse import bass2jax, mybir

    nc = _build_nc(niter, ablate)
    bass2jax.install_neuronx_cc_hook()

    partition_name = (
        nc.partition_id_tensor.name if nc.partition_id_tensor else None)
    in_names, out_names, out_avals, zero_shapes = [], [], [], []
    for alloc in nc.m.functions[0].allocations:
        if not isinstance(alloc, mybir.MemoryLocationSet):
            continue
        name = alloc.memorylocations[0].name
        if alloc.kind == "ExternalInput":
            if name != partition_name:
                in_names.append(name)
        elif alloc.kind == "ExternalOutput":
            out_names.append(name)
            shape = tuple(alloc.tensor_shape)
            dtype = mybir.dt.np(alloc.dtype)
            out_avals.append(jax.core.ShapedArray(shape, dtype))
            zero_shapes.append((shape, dtype))
    n_params = len(in_names)
    n_outs = len(out_avals)
    all_names = list(in_names) + list(out_names)
    if partition_name is not None:
        all_names.append(partition_name)
    donate = tuple(range(n_params, n_params + n_outs))

    def _body(*args):
        operands = list(args)
        if partition_name is not None:
            operands.append(bass2jax.partition_id_tensor())
        outs = bass2jax._bass_exec_p.bind(
            *operands,
            out_avals=tuple(out_avals),
            in_names=tuple(all_names),
            out_names=tuple(out_names),
            lowering_input_output_aliases=(),
            sim_require_finite=True,
            sim_require_nnan=True,
            nc=nc,
        )
        return tuple(outs)

    devices = jax.devices()[:NCORES]
    mesh = Mesh(np.asarray(devices), ("core",))
    in_specs = (PartitionSpec("core"),) * (n_params + n_outs)
    out_specs = (PartitionSpec("core"),) * n_outs
    sharded = jax.jit(
        shard_map(_body, mesh=mesh, in_specs=in_specs, out_specs=out_specs,
                  check_rep=False),
        donate_argnums=donate,
        keep_unused=True,
    )
    sharding = NamedSharding(mesh, PartitionSpec("core"))

    def put_inputs(in_maps):
        return [
            jax.device_put(
                np.concatenate(
                    [np.asarray(in_maps[c][nm]) for c in range(NCORES)], axis=0),
                sharding)
            for nm in in_names
        ]

    def run(in_maps, device_inputs=None):
        if device_inputs is None:
            device_inputs = put_inputs(in_maps)
        zeros = [
            jax.device_put(np.zeros((NCORES * s[0], *s[1:]), d), sharding)
            for s, d in zero_shapes
        ]
        out_arrs = sharded(*device_inputs, *zeros)
        results = []
        for c in range(NCORES):
            results.append({
                name: np.asarray(out_arrs[i]).reshape(
                    NCORES, *out_avals[i].shape)[c]
                for i, name in enumerate(out_names)
            })
        return results

    runner = {"run": run, "put_inputs": put_inputs, "sharded": sharded,
              "in_names": in_names, "out_names": out_names,
              "zero_shapes": zero_shapes, "nc": nc}
    _STATE[key] = runner
    return runner


def kernel(Q=None, K=None, V=None, W_H=None, b_H=None, mask=None, **kw):
    in_maps = _host_shard(Q, K, V, W_H, b_H)
    runner = _get_runner(niter=1)
    results = runner["run"](in_maps)
    out = np.empty((B, L, D), np.float32)
    for c in range(NCORES):
        b, half = divmod(c, 2)
        out[b, half * QBLK:(half + 1) * QBLK, :] = results[c]["out"]
    return out
